# revision 1
# baseline (speedup 1.0000x reference)
"""Trainium2 Bass kernel for DenseRoutingMaskLayer (MoE routing chunk-gather).

reference: route = argmax(routing_inputs, -1); out[b] = inputs[b].reshape(8, 512)[route[b]]

Pure data parallel across 8 NeuronCores (2048 rows each). Per core, raw-bacc
program with explicit per-engine pipelines over 4 row-blocks:

  ACT : loads routing in its natural layout (partition p = i//16, so every
        descriptor moves a contiguous 512B run), plus weight/identity tiles;
        later stores odd halves of gathered blocks (2nd HWDGE ring)
  DVE : argmax of the 8 logits via a max tree, then the flat gather index
        idx(i) = 8*i + route(i) exactly in f32 as sum(one_hot * (r + 8i));
        int16 pack via bitcast, and a stream_shuffle that replicates the
        wrapped index pattern to partitions 16..31
  PE  : transposes the [rows/16, 16] index tile into the gather's wrapped
        [16, rows/16] layout (matmul against a block identity)
  POOL: gpsimd.dma_gather pulls only the selected 512-float chunk per row
        (4 MiB/core instead of 32 MiB/core dense), issued as 8 sub-gathers
        of 256 rows so stores unblock at 0.5 MiB granularity; the mlp Q7
        library is loaded explicitly up front so its ~9us fetch overlaps
        the prologue
  SP  : stores even sub-blocks (1st HWDGE ring)

The gathered row i lands at SBUF partition i%128, col i//128; stores use a
matching strided DRAM view. Index tile partitions 32..127 are memset to 0
(the queue-0 gather ucode only reads partitions 0..31).
"""

import sys

import numpy as np

try:
    import concourse  # noqa: F401
except ImportError:  # pragma: no cover
    sys.path.insert(0, "/opt/trn_rl_repo")

N_CORES = 8
B_FULL = 16384
D = 4096
ROUTES = 8
RW = D // ROUTES
B_SH = B_FULL // N_CORES  # 2048
NC_COLS = B_SH // 16  # 128
NJ = B_SH // 128  # 16
NB = 4

_prog_cache = {}


def _build_program(nb=NB):
    import concourse.bacc as bacc
    import concourse.mybir as mybir
    from concourse.library_config import mlp
    from contextlib import ExitStack

    f32 = mybir.dt.float32
    i32 = mybir.dt.int32
    i16 = mybir.dt.int16
    Alu = mybir.AluOpType

    jb = NJ // nb
    cb = NC_COLS // nb  # wrapped cols per block
    pb = cb  # natural-layout partitions per block (= rows/16, same count)
    rows_b = 128 * jb

    nc = bacc.Bacc("TRN2", target_bir_lowering=False, debug=False, num_devices=N_CORES)
    x = nc.dram_tensor("x", [B_SH, D], f32, kind="ExternalInput")
    rt = nc.dram_tensor("rt", [B_SH, ROUTES], f32, kind="ExternalInput")
    wt = nc.dram_tensor("wt", [128, 16, ROUTES], f32, kind="ExternalInput")
    it = nc.dram_tensor("it", [128, pb], f32, kind="ExternalInput")
    # partition-major output: y[p, j, :] holds row j*128+p; the host
    # transposes back. Keeps every store descriptor 4KB-contiguous.
    y = nc.dram_tensor("y", [128, NJ, RW], f32, kind="ExternalOutput")

    x_rows = x.ap().rearrange("b (r w) -> (b r) w", r=ROUTES)
    rt_n = rt.ap().rearrange("(p u) r -> p u r", u=16)  # [128, 16, 8]
    y_pjw = y.ap()

    with (
        ExitStack() as ctx,
        nc.sbuf_tensor("wt_t", [128, 16, ROUTES], f32) as wt_t,
        nc.sbuf_tensor("it_t", [128, pb], f32) as it_t,
        nc.sbuf_tensor("r_t", [128, 16, ROUTES], f32) as r_t,
        nc.sbuf_tensor("mx4", [128, 16, 4], f32) as mx4,
        nc.sbuf_tensor("mx2", [128, 16, 2], f32) as mx2,
        nc.sbuf_tensor("m_t", [128, 16], f32) as m_t,
        nc.sbuf_tensor("eq_t", [128, 16, ROUTES], f32) as eq_t,
        nc.sbuf_tensor("s4", [128, 16, 4], f32) as s4,
        nc.sbuf_tensor("s2", [128, 16, 2], f32) as s2,
        nc.sbuf_tensor("idf", [128, 16], f32) as idf,
        nc.sbuf_tensor("idx32", [32, NC_COLS], i32) as idx32,
        nc.sbuf_tensor("idx16", [128, NC_COLS], i16) as idx16,
        nc.sbuf_tensor("g_t", [128, NJ, RW], f32) as g_t,
        nc.Block(no_gpsimd_drain=True) as block,
    ):
        t1 = [
            ctx.enter_context(nc.psum_tensor(f"t1_{b}", [16, pb], f32))
            for b in range(nb)
        ]
        s_rt = [ctx.enter_context(nc.semaphore(f"s_rt{b}")) for b in range(nb)]
        s_wt = ctx.enter_context(nc.semaphore("s_wt"))
        s_id = ctx.enter_context(nc.semaphore("s_id"))
        s_v = ctx.enter_context(nc.semaphore("s_v"))
        s_mm = ctx.enter_context(nc.semaphore("s_mm"))
        s_g = [ctx.enter_context(nc.semaphore(f"s_g{k}")) for k in range(2 * nb)]
        s_y = ctx.enter_context(nc.semaphore("s_y"))

        OPB = 10  # DVE ops per block
        OFF = 2  # the two leading memsets

        @block.scalar
        def _(act):
            act.dma_start(r_t[0:pb, :, :], rt_n[0:pb, :, :]).then_inc(s_rt[0], 16)
            act.dma_start(wt_t[:], wt.ap()).then_inc(s_wt, 16)
            act.dma_start(it_t[:], it.ap()).then_inc(s_id, 16)
            for b in range(1, nb):
                ps = slice(b * pb, (b + 1) * pb)
                act.dma_start(r_t[ps, :, :], rt_n[ps, :, :]).then_inc(s_rt[b], 16)
            hj = jb // 2
            for k in range(1, 2 * nb, 2):
                js = slice(k * hj, (k + 1) * hj)
                act.wait_ge(s_g[k], 16)
                act.dma_start(y_pjw[:, js, :], g_t[:, js, :]).then_inc(s_y, 16)

        @block.vector
        def _(dve):
            k = 0

            def step(inst):
                nonlocal k
                k += 1
                inst.then_inc(s_v, 1)
                dve.wait_ge(s_v, k)

            step(dve.memset(idx32[:], 0))
            step(dve.memset(idx16[:], 0))
            dve.wait_ge(s_wt, 16)
            shuffle_mask = list(range(16)) * 2
            for b in range(nb):
                ps = slice(b * pb, (b + 1) * pb)
                cs = slice(b * cb, (b + 1) * cb)
                dve.wait_ge(s_rt[b], 16)
                r = r_t[ps, :, :]
                step(dve.tensor_tensor(mx4[ps], r[:, :, 0:4], r[:, :, 4:8], Alu.max))
                step(
                    dve.tensor_tensor(
                        mx2[ps], mx4[ps, :, 0:2], mx4[ps, :, 2:4], Alu.max
                    )
                )
                step(dve.tensor_tensor(m_t[ps], mx2[ps, :, 0], mx2[ps, :, 1], Alu.max))
                step(
                    dve.tensor_tensor(
                        eq_t[ps],
                        r,
                        m_t[ps].unsqueeze(2).broadcast_to([pb, 16, ROUTES]),
                        Alu.is_equal,
                    )
                )
                step(dve.tensor_tensor(eq_t[ps], eq_t[ps], wt_t[ps], Alu.mult))
                step(
                    dve.tensor_tensor(
                        s4[ps], eq_t[ps, :, 0:4], eq_t[ps, :, 4:8], Alu.add
                    )
                )
                step(dve.tensor_tensor(s2[ps], s4[ps, :, 0:2], s4[ps, :, 2:4], Alu.add))
                step(dve.tensor_tensor(idf[ps], s2[ps, :, 0], s2[ps, :, 1], Alu.add))
                dve.wait_ge(s_mm, b + 1)
                step(dve.tensor_copy(idx32[0:16, cs], t1[b][:]))
                step(
                    dve.stream_shuffle(
                        idx16[0:32, cs],
                        idx32[0:32, cs]
                        .bitcast(i16)
                        .rearrange("q (c two) -> q c two", two=2)[:, :, 0],
                        shuffle_mask,
                    )
                )

        @block.tensor
        def _(pe):
            pe.wait_ge(s_id, 16)
            for b in range(nb):
                ps = slice(b * pb, (b + 1) * pb)
                pe.wait_ge(s_v, OFF + OPB * b + 8)
                pe.transpose(
                    t1[b][:], idf[ps], it_t[ps, 0:pb], tile_position=(b * pb, 0)
                ).then_inc(s_mm, 1)

        @block.gpsimd
        def _(pool):
            pool.load_library(mlp)
            hc, hj = cb // 2, jb // 2
            rows_h = rows_b // 2
            for b in range(nb):
                pool.wait_ge(s_v, OFF + OPB * (b + 1))
                for h in range(2):
                    k = 2 * b + h
                    cs = slice(k * hc, (k + 1) * hc)
                    js = slice(k * hj, (k + 1) * hj)
                    pool.dma_gather(
                        g_t[:, js, :], x_rows, idx16[:, cs], rows_h, rows_h, RW,
                        single_packet=False,
                    ).then_inc(s_g[k], 16)

        @block.sync
        def _(sp):
            hj = jb // 2
            for k in range(0, 2 * nb, 2):
                js = slice(k * hj, (k + 1) * hj)
                sp.wait_ge(s_g[k], 16)
                sp.dma_start(y_pjw[:, js, :], g_t[:, js, :]).then_inc(s_y, 16)
            sp.wait_ge(s_y, 32 * nb)

    nc.compile()
    return nc


def _get_program(nb=NB):
    if nb not in _prog_cache:
        _prog_cache[nb] = _build_program(nb)
    return _prog_cache[nb]


def _weights():
    p = np.arange(128, dtype=np.float32)[:, None, None]
    u = np.arange(16, dtype=np.float32)[None, :, None]
    r = np.arange(ROUTES, dtype=np.float32)[None, None, :]
    return np.ascontiguousarray(r + 8.0 * (p * 16.0 + u), dtype=np.float32)


def _identity(nb=NB):
    pb = NC_COLS // nb
    p = np.arange(128)[:, None]
    j = np.arange(pb)[None, :]
    return np.ascontiguousarray((p % pb == j).astype(np.float32))


def kernel(inputs: np.ndarray, routing_inputs: np.ndarray) -> np.ndarray:
    from concourse.bass_utils import run_bass_kernel_spmd

    inputs = np.ascontiguousarray(inputs, dtype=np.float32)
    routing_inputs = np.ascontiguousarray(routing_inputs, dtype=np.float32)
    wt = _weights()
    it = _identity()
    nc = _get_program()
    in_maps = [
        {
            "x": inputs[c * B_SH : (c + 1) * B_SH],
            "rt": routing_inputs[c * B_SH : (c + 1) * B_SH],
            "wt": wt,
            "it": it,
        }
        for c in range(N_CORES)
    ]
    res = None
    for attempt in range(3):
        try:
            res = run_bass_kernel_spmd(nc, in_maps, core_ids=list(range(N_CORES)))
            break
        except Exception:  # transient NRT_EXEC_UNIT_UNRECOVERABLE flakes
            if attempt == 2:
                raise
            import time

            time.sleep(2.0)
    return np.concatenate(
        [
            res.results[c]["y"].transpose(1, 0, 2).reshape(B_SH, RW)
            for c in range(N_CORES)
        ],
        axis=0,
    )

